# revision 2
# baseline (speedup 1.0000x reference)
"""Trainium2 Bass kernel for nn_EpisodicMemoryEfficient.

Computation (see reference): per batch, x = states reshaped [S, 512];
q/k/v = x @ {Wq,Wk,Wv}.T; chunked sliding-window attention with chunk size
C=64 where chunk i attends to chunks {i-1, i} with strict causal masking
(chunk 0 has no previous chunk), softmax over the 128-key context, out = p@v.

Sharding: pure data parallel over (batch, sequence-half) -> 8 cores.
Each core receives a pre-transposed xT [512, 4224] covering its 4096-row
shard plus a 128-row halo in front (zeros for the first half, the real
previous rows for the second half).  Weights are passed transposed and
replicated; masks are tiny per-core constants.

Device kernel layout choices (all fp32):
  - qT/kT are produced directly in [d, seq] orientation (weights stationary,
    xT moving, N=512 matmuls); v in natural [seq, d] (xT stationary).
  - attention uses the transposed-scores formulation sT[j, qi] so that the
    softmax denominator comes from a matmul-with-ones column sum and the
    PV matmul needs no transpose at all (lhsT = exp(sT), rhs = v natural).
  - masking is additive (-1e30) on the [j, qi] tile before exp; the scale
    1/sqrt(512) is folded into the ACT exp instruction.
  - PSUM is evacuated with an ACT copy scaled by the per-partition 1/rowsum.
The only fully-masked row (global position 0) yields 0/0 -> NaN and is
overwritten with zeros on the host, matching the reference.
"""

import numpy as np

_D = 512
_C = 64
_HALO = 128  # two chunks of halo keeps everything 128-aligned
_NEG = -1.0e30
_SCALE = 1.0 / float(np.sqrt(_D))

_BUILD_CACHE = {}


def _build(L):
    """Build the per-core Bass kernel for a shard of L query rows."""
    from contextlib import ExitStack

    import concourse.bass as bass
    import concourse.mybir as mybir
    import concourse.tile as tile
    from concourse import bacc

    assert L % 128 == 0 and L >= 128
    NX = L + _HALO          # x rows held on-core (halo + shard)
    NT = NX // 128          # 128-row x-tiles
    NQ = L // 128           # q-tiles
    dt = mybir.dt.float32

    nc = bacc.Bacc("TRN2", target_bir_lowering=False, debug=False, num_devices=8)

    xT_d = nc.dram_tensor("xT", [_D, NX], dt, kind="ExternalInput")
    wqt_d = nc.dram_tensor("wqt", [_D, _D], dt, kind="ExternalInput")
    wkt_d = nc.dram_tensor("wkt", [_D, _D], dt, kind="ExternalInput")
    wvt_d = nc.dram_tensor("wvt", [_D, _D], dt, kind="ExternalInput")
    mask0_d = nc.dram_tensor("mask0", [128, 256], dt, kind="ExternalInput")
    maskr_d = nc.dram_tensor("maskr", [128, 256], dt, kind="ExternalInput")
    out_d = nc.dram_tensor("out", [L, _D], dt, kind="ExternalOutput")

    # projection blocks over the x column range: 512 wide, 128-wide leftover
    blocks = []
    m0 = 0
    while m0 < NX:
        mb = min(512, NX - m0)
        blocks.append((m0, mb))
        m0 += mb

    with ExitStack() as ctx:
        tc = ctx.enter_context(tile.TileContext(nc))
        const = ctx.enter_context(tc.tile_pool(name="const", bufs=1))
        xpool = ctx.enter_context(tc.tile_pool(name="xpool", bufs=2))
        qpool = ctx.enter_context(tc.tile_pool(name="qpool", bufs=3))
        kpool = ctx.enter_context(tc.tile_pool(name="kpool", bufs=3))
        vpool = ctx.enter_context(tc.tile_pool(name="vpool", bufs=3))
        spool = ctx.enter_context(tc.tile_pool(name="spool", bufs=3))
        opool = ctx.enter_context(tc.tile_pool(name="opool", bufs=3))
        ps_proj = ctx.enter_context(tc.tile_pool(name="ps_proj", bufs=3, space="PSUM"))
        ps_att = ctx.enter_context(tc.tile_pool(name="ps_att", bufs=2, space="PSUM"))
        ps_out = ctx.enter_context(tc.tile_pool(name="ps_out", bufs=2, space="PSUM"))
        ps_sum = ctx.enter_context(tc.tile_pool(name="ps_sum", bufs=1, space="PSUM"))

        # ---- constants ----
        wq_sb, wk_sb, wv_sb = [], [], []
        for c in range(4):
            for lst, dram, nm in ((wq_sb, wqt_d, "wq"), (wk_sb, wkt_d, "wk"),
                                  (wv_sb, wvt_d, "wv")):
                t_ = const.tile([128, _D], dt, tag=f"{nm}{c}", name=f"{nm}{c}")
                nc.sync.dma_start(out=t_, in_=dram[128 * c:128 * (c + 1), :])
                lst.append(t_)
        m0_sb = const.tile([128, 256], dt, tag="m0", name="m0_sb")
        nc.sync.dma_start(out=m0_sb, in_=mask0_d[:, :])
        mr_sb = const.tile([128, 256], dt, tag="mr", name="mr_sb")
        nc.sync.dma_start(out=mr_sb, in_=maskr_d[:, :])
        ones_sb = const.tile([128, 1], dt, tag="ones", name="ones_sb")
        nc.vector.memset(ones_sb, 1.0)

        qt_tiles = {}   # (block, c) -> [128, mb] tile, cols = x-tile cols
        kt_tiles = {}
        v_tiles = {}    # x-tile index -> [128, 512] natural tile

        def project_block(bi):
            m0, mb = blocks[bi]
            xt = []
            for c in range(4):
                t_ = xpool.tile([128, mb], dt, tag=f"xt{c}", name=f"xt{c}_b{bi}")
                nc.sync.dma_start(out=t_, in_=xT_d[128 * c:128 * (c + 1), m0:m0 + mb])
                xt.append(t_)
            for o in range(4):
                psq = ps_proj.tile([128, mb], dt, tag="ps_proj", name=f"psq{o}_b{bi}")
                for c in range(4):
                    nc.tensor.matmul(psq, wq_sb[c][:, 128 * o:128 * (o + 1)], xt[c],
                                     start=(c == 0), stop=(c == 3))
                qt = qpool.tile([128, mb], dt, tag=f"qt{o}", name=f"qt{o}_b{bi}")
                nc.vector.tensor_copy(qt, psq)
                qt_tiles[(bi, o)] = qt

                psk = ps_proj.tile([128, mb], dt, tag="ps_proj", name=f"psk{o}_b{bi}")
                for c in range(4):
                    nc.tensor.matmul(psk, wk_sb[c][:, 128 * o:128 * (o + 1)], xt[c],
                                     start=(c == 0), stop=(c == 3))
                kt = kpool.tile([128, mb], dt, tag=f"kt{o}", name=f"kt{o}_b{bi}")
                nc.vector.tensor_copy(kt, psk)
                kt_tiles[(bi, o)] = kt
            for s in range(mb // 128):
                ti = m0 // 128 + s
                psv = ps_proj.tile([128, _D], dt, tag="ps_proj", name=f"psv{s}_b{bi}")
                for c in range(4):
                    nc.tensor.matmul(psv, xt[c][:, 128 * s:128 * (s + 1)], wv_sb[c],
                                     start=(c == 0), stop=(c == 3))
                vt = vpool.tile([128, _D], dt, tag=f"v{ti % 4}", name=f"v_t{ti}")
                nc.scalar.copy(vt, psv)
                v_tiles[ti] = vt

        def attend(t):
            # q = x-tile t+1; keys region B = x-tile t+1, region A = x-tile t
            bq, cq = (t + 1) // 4, ((t + 1) % 4) * 128
            ba, ca = t // 4, (t % 4) * 128
            ps_s = ps_att.tile([128, 256], dt, tag="ps_s", name=f"ps_s_t{t}")
            for c in range(4):  # region B -> cols 0:128 of ps_s
                nc.tensor.matmul(ps_s[:, 0:128],
                                 kt_tiles[(bq, c)][:, cq:cq + 128],
                                 qt_tiles[(bq, c)][:, cq:cq + 128],
                                 start=(c == 0), stop=(c == 3))
            for c in range(4):  # region A -> cols 128:256
                nc.tensor.matmul(ps_s[:, 128:256],
                                 kt_tiles[(ba, c)][:, ca:ca + 128],
                                 qt_tiles[(bq, c)][:, cq:cq + 128],
                                 start=(c == 0), stop=(c == 3))
            s_sb = spool.tile([128, 256], dt, tag="s_sb", name=f"s_t{t}")
            nc.vector.tensor_add(s_sb, ps_s, m0_sb if t == 0 else mr_sb)
            e_sb = spool.tile([128, 256], dt, tag="e_sb", name=f"e_t{t}")
            nc.scalar.activation(e_sb, s_sb, mybir.ActivationFunctionType.Exp,
                                 scale=_SCALE)
            eB, eA = e_sb[:, 0:128], e_sb[:, 128:256]
            psum_t = ps_sum.tile([128, 1], dt, tag="ps_sum", name=f"pssum_t{t}")
            po = ps_out.tile([128, _D], dt, tag="ps_o", name=f"ps_o_t{t}")
            nc.tensor.matmul(psum_t, eA, ones_sb, start=True, stop=False)
            nc.tensor.matmul(po, eA, v_tiles[t], start=True, stop=False)
            nc.tensor.matmul(psum_t, eB, ones_sb, start=False, stop=True)
            nc.tensor.matmul(po, eB, v_tiles[t + 1], start=False, stop=True)
            recip = opool.tile([128, 1], dt, tag="recip", name=f"recip_t{t}")
            nc.vector.reciprocal(recip, psum_t)
            o_sb = opool.tile([128, _D], dt, tag="o_sb", name=f"o_t{t}")
            nc.scalar.mul(o_sb, po, recip)
            nc.sync.dma_start(out=out_d[128 * t:128 * (t + 1), :], in_=o_sb)

        for bi in range(len(blocks)):
            project_block(bi)
            # attention tiles whose newest dependency is block bi
            for t in range(max(0, 4 * bi - 1), min(NQ, 4 * bi + 3)):
                if (t + 1) // 4 == bi:
                    attend(t)

    nc.compile()
    return nc


def _get_nc(L):
    if L not in _BUILD_CACHE:
        _BUILD_CACHE[L] = _build(L)
    return _BUILD_CACHE[L]


def _masks():
    p = np.arange(128)[:, None]
    q = np.arange(128)[None, :]
    allow_b = p < q                      # region B: keys in the query's own x-tile
    allow_a = (p >= 64) & (q < 64)       # region A: previous chunk only
    maskr = np.concatenate([np.where(allow_b, 0.0, _NEG),
                            np.where(allow_a, 0.0, _NEG)], axis=1).astype(np.float32)
    mask0 = np.concatenate([np.where(allow_b, 0.0, _NEG),
                            np.full((128, 128), _NEG)], axis=1).astype(np.float32)
    return maskr, mask0


def _make_in_maps(states, Wq, Wk, Wv):
    states = np.ascontiguousarray(np.asarray(states, dtype=np.float32))
    B, S = states.shape[0], states.shape[1]
    x = states.reshape(B, S, _D)
    L = S // 2
    NX = L + _HALO
    wqt = np.ascontiguousarray(np.asarray(Wq, dtype=np.float32).T)
    wkt = np.ascontiguousarray(np.asarray(Wk, dtype=np.float32).T)
    wvt = np.ascontiguousarray(np.asarray(Wv, dtype=np.float32).T)
    maskr, mask0 = _masks()
    in_maps = []
    for core in range(2 * B):
        b, h = core // 2, core % 2
        xp = np.zeros((NX, _D), dtype=np.float32)
        lo = h * L - _HALO
        if lo < 0:
            xp[_HALO:] = x[b, 0:L]
        else:
            xp[:] = x[b, lo:lo + NX]
        in_maps.append({
            "xT": np.ascontiguousarray(xp.T),
            "wqt": wqt, "wkt": wkt, "wvt": wvt,
            "mask0": mask0 if h == 0 else maskr,
            "maskr": maskr,
        })
    return in_maps


def _assemble(results, B, S):
    L = S // 2
    out = np.empty((B, S, _D), dtype=np.float32)
    for core, res in enumerate(results):
        b, h = core // 2, core % 2
        out[b, h * L:(h + 1) * L] = res["out"]
    out[:, 0, :] = 0.0
    return out.reshape(B, S, _D // 2, 2)


def run(states, Wq, Wk, Wv, trace=False):
    """Run on 8 NeuronCores; returns (output, BassKernelResults)."""
    from concourse.bass_utils import run_bass_kernel_spmd

    states = np.asarray(states)
    B, S = states.shape[0], states.shape[1]
    assert B == 4 and S % 128 == 0
    nc = _get_nc(S // 2)
    in_maps = _make_in_maps(states, Wq, Wk, Wv)
    try:
        res = run_bass_kernel_spmd(nc, in_maps, core_ids=list(range(8)), trace=trace)
    except ModuleNotFoundError:
        # axon NTFF hook unavailable in this container — run untraced
        res = run_bass_kernel_spmd(nc, in_maps, core_ids=list(range(8)), trace=False)
    return _assemble(res.results, B, S), res


def kernel(states, Wq, Wk, Wv):
    out, _ = run(states, Wq, Wk, Wv, trace=False)
    return out


# revision 4
# speedup vs baseline: 171.0568x; 171.0568x over previous
"""Trainium2 Bass kernel for nn_EpisodicMemoryEfficient.

Computation (see reference): per batch, x = states reshaped [S, 512];
q/k/v = x @ {Wq,Wk,Wv}.T; chunked sliding-window attention with chunk size
C=64 where chunk i attends to chunks {i-1, i} with strict causal masking
(chunk 0 has no previous chunk), softmax over the 128-key context, out = p@v.

Sharding: pure data parallel over (batch, sequence-half) -> 8 cores.
Each core receives a pre-transposed xT [512, 4224] covering its 4096-row
shard plus a 128-row halo in front (zeros for the first half, the real
previous rows for the second half).  Weights are passed transposed and
replicated; masks are tiny per-core constants.

Device kernel layout choices (all fp32):
  - qT/kT are produced directly in [d, seq] orientation (weights stationary,
    xT moving, N=512 matmuls); v in natural [seq, d] (xT stationary).
  - attention uses the transposed-scores formulation sT[j, qi] so that the
    softmax denominator comes from a matmul-with-ones column sum and the
    PV matmul needs no transpose at all (lhsT = exp(sT), rhs = v natural).
  - masking is additive (-1e30) on the [j, qi] tile before exp; the scale
    1/sqrt(512) is folded into the ACT exp instruction.
  - PSUM is evacuated with an ACT copy scaled by the per-partition 1/rowsum.
The only fully-masked row (global position 0) yields 0/0 -> NaN and is
overwritten with zeros on the host, matching the reference.
"""

import numpy as np

_D = 512
_C = 64
_HALO = 128  # two chunks of halo keeps everything 128-aligned
_NEG = -1.0e30
_SCALE = 1.0 / float(np.sqrt(_D))

_BUILD_CACHE = {}


def _build(L, repeat=1):
    """Build the per-core Bass kernel for a shard of L query rows.

    repeat>1 wraps the whole body in a For_i loop — used only for
    wall-clock timing (amortizes the ~100ms axon dispatch overhead).
    """
    from contextlib import ExitStack

    import concourse.bass as bass
    import concourse.mybir as mybir
    import concourse.tile as tile
    from concourse import bacc

    assert L % 128 == 0 and L >= 128
    NX = L + _HALO          # x rows held on-core (halo + shard)
    NT = NX // 128          # 128-row x-tiles
    NQ = L // 128           # q-tiles
    dt = mybir.dt.float32

    nc = bacc.Bacc("TRN2", target_bir_lowering=False, debug=False, num_devices=8)

    xT_d = nc.dram_tensor("xT", [_D, NX], dt, kind="ExternalInput")
    wqt_d = nc.dram_tensor("wqt", [_D, _D], dt, kind="ExternalInput")
    wkt_d = nc.dram_tensor("wkt", [_D, _D], dt, kind="ExternalInput")
    wvt_d = nc.dram_tensor("wvt", [_D, _D], dt, kind="ExternalInput")
    mask0_d = nc.dram_tensor("mask0", [128, 256], dt, kind="ExternalInput")
    maskr_d = nc.dram_tensor("maskr", [128, 256], dt, kind="ExternalInput")
    out_d = nc.dram_tensor("out", [L, _D], dt, kind="ExternalOutput")

    # projection blocks over the x column range: 512 wide, 128-wide leftover
    blocks = []
    m0 = 0
    while m0 < NX:
        mb = min(512, NX - m0)
        blocks.append((m0, mb))
        m0 += mb

    with ExitStack() as ctx:
        tc = ctx.enter_context(tile.TileContext(nc))
        const = ctx.enter_context(tc.tile_pool(name="const", bufs=1))
        xpool = ctx.enter_context(tc.tile_pool(name="xpool", bufs=2))
        qpool = ctx.enter_context(tc.tile_pool(name="qpool", bufs=3))
        kpool = ctx.enter_context(tc.tile_pool(name="kpool", bufs=3))
        vpool = ctx.enter_context(tc.tile_pool(name="vpool", bufs=3))
        spool = ctx.enter_context(tc.tile_pool(name="spool", bufs=3))
        opool = ctx.enter_context(tc.tile_pool(name="opool", bufs=3))
        ps_proj = ctx.enter_context(tc.tile_pool(name="ps_proj", bufs=3, space="PSUM"))
        ps_att = ctx.enter_context(tc.tile_pool(name="ps_att", bufs=2, space="PSUM"))
        ps_out = ctx.enter_context(tc.tile_pool(name="ps_out", bufs=2, space="PSUM"))
        ps_sum = ctx.enter_context(tc.tile_pool(name="ps_sum", bufs=1, space="PSUM"))

        # ---- constants ----
        wq_sb, wk_sb, wv_sb = [], [], []
        for c in range(4):
            for lst, dram, nm in ((wq_sb, wqt_d, "wq"), (wk_sb, wkt_d, "wk"),
                                  (wv_sb, wvt_d, "wv")):
                t_ = const.tile([128, _D], dt, tag=f"{nm}{c}", name=f"{nm}{c}")
                nc.sync.dma_start(out=t_, in_=dram[128 * c:128 * (c + 1), :])
                lst.append(t_)
        m0_sb = const.tile([128, 256], dt, tag="m0", name="m0_sb")
        nc.sync.dma_start(out=m0_sb, in_=mask0_d[:, :])
        mr_sb = const.tile([128, 256], dt, tag="mr", name="mr_sb")
        nc.sync.dma_start(out=mr_sb, in_=maskr_d[:, :])
        ones_sb = const.tile([128, 1], dt, tag="ones", name="ones_sb")
        nc.vector.memset(ones_sb, 1.0)

        qt_tiles = {}   # (block, c) -> [128, mb] tile, cols = x-tile cols
        kt_tiles = {}
        v_tiles = {}    # x-tile index -> [128, 512] natural tile

        def project_block(bi):
            m0, mb = blocks[bi]
            xt = []
            for c in range(4):
                t_ = xpool.tile([128, mb], dt, tag=f"xt{c}", name=f"xt{c}_b{bi}")
                nc.sync.dma_start(out=t_, in_=xT_d[128 * c:128 * (c + 1), m0:m0 + mb])
                xt.append(t_)
            for o in range(4):
                psq = ps_proj.tile([128, mb], dt, tag="ps_proj", name=f"psq{o}_b{bi}")
                for c in range(4):
                    nc.tensor.matmul(psq, wq_sb[c][:, 128 * o:128 * (o + 1)], xt[c],
                                     start=(c == 0), stop=(c == 3))
                qt = qpool.tile([128, mb], dt, tag=f"qt{o}", name=f"qt{o}_b{bi}")
                nc.vector.tensor_copy(qt, psq)
                qt_tiles[(bi, o)] = qt

                psk = ps_proj.tile([128, mb], dt, tag="ps_proj", name=f"psk{o}_b{bi}")
                for c in range(4):
                    nc.tensor.matmul(psk, wk_sb[c][:, 128 * o:128 * (o + 1)], xt[c],
                                     start=(c == 0), stop=(c == 3))
                kt = kpool.tile([128, mb], dt, tag=f"kt{o}", name=f"kt{o}_b{bi}")
                nc.vector.tensor_copy(kt, psk)
                kt_tiles[(bi, o)] = kt
            for s in range(mb // 128):
                ti = m0 // 128 + s
                psv = ps_proj.tile([128, _D], dt, tag="ps_proj", name=f"psv{s}_b{bi}")
                for c in range(4):
                    nc.tensor.matmul(psv, xt[c][:, 128 * s:128 * (s + 1)], wv_sb[c],
                                     start=(c == 0), stop=(c == 3))
                vt = vpool.tile([128, _D], dt, tag=f"v{ti % 4}", name=f"v_t{ti}")
                nc.scalar.copy(vt, psv)
                v_tiles[ti] = vt

        def attend(t):
            # q = x-tile t+1; keys region B = x-tile t+1, region A = x-tile t
            bq, cq = (t + 1) // 4, ((t + 1) % 4) * 128
            ba, ca = t // 4, (t % 4) * 128
            ps_s = ps_att.tile([128, 256], dt, tag="ps_s", name=f"ps_s_t{t}")
            for c in range(4):  # region B -> cols 0:128 of ps_s
                nc.tensor.matmul(ps_s[:, 0:128],
                                 kt_tiles[(bq, c)][:, cq:cq + 128],
                                 qt_tiles[(bq, c)][:, cq:cq + 128],
                                 start=(c == 0), stop=(c == 3))
            for c in range(4):  # region A -> cols 128:256
                nc.tensor.matmul(ps_s[:, 128:256],
                                 kt_tiles[(ba, c)][:, ca:ca + 128],
                                 qt_tiles[(bq, c)][:, cq:cq + 128],
                                 start=(c == 0), stop=(c == 3))
            s_sb = spool.tile([128, 256], dt, tag="s_sb", name=f"s_t{t}")
            nc.vector.tensor_add(s_sb, ps_s, m0_sb if t == 0 else mr_sb)
            e_sb = spool.tile([128, 256], dt, tag="e_sb", name=f"e_t{t}")
            nc.scalar.activation(e_sb, s_sb, mybir.ActivationFunctionType.Exp,
                                 scale=_SCALE)
            eB, eA = e_sb[:, 0:128], e_sb[:, 128:256]
            psum_t = ps_sum.tile([128, 1], dt, tag="ps_sum", name=f"pssum_t{t}")
            po = ps_out.tile([128, _D], dt, tag="ps_o", name=f"ps_o_t{t}")
            nc.tensor.matmul(psum_t, eA, ones_sb, start=True, stop=False)
            nc.tensor.matmul(po, eA, v_tiles[t], start=True, stop=False)
            nc.tensor.matmul(psum_t, eB, ones_sb, start=False, stop=True)
            nc.tensor.matmul(po, eB, v_tiles[t + 1], start=False, stop=True)
            recip = opool.tile([128, 1], dt, tag="recip", name=f"recip_t{t}")
            nc.vector.reciprocal(recip, psum_t)
            o_sb = opool.tile([128, _D], dt, tag="o_sb", name=f"o_t{t}")
            nc.scalar.mul(o_sb, po, recip)
            nc.sync.dma_start(out=out_d[128 * t:128 * (t + 1), :], in_=o_sb)

        def body():
            for bi in range(len(blocks)):
                project_block(bi)
                # attention tiles whose newest dependency is block bi
                for t in range(max(0, 4 * bi - 1), min(NQ, 4 * bi + 3)):
                    if (t + 1) // 4 == bi:
                        attend(t)

        if repeat == 1:
            body()
        else:
            import concourse.mybir as _mb
            with tc.For_i(0, repeat, 1,
                          hint_engines=(_mb.EngineType.PE, _mb.EngineType.DVE,
                                        _mb.EngineType.Activation,
                                        _mb.EngineType.SP)):
                body()

    nc.compile()
    return nc


def _get_nc(L):
    if L not in _BUILD_CACHE:
        _BUILD_CACHE[L] = _build(L)
    return _BUILD_CACHE[L]


def _masks():
    p = np.arange(128)[:, None]
    q = np.arange(128)[None, :]
    allow_b = p < q                      # region B: keys in the query's own x-tile
    allow_a = (p >= 64) & (q < 64)       # region A: previous chunk only
    maskr = np.concatenate([np.where(allow_b, 0.0, _NEG),
                            np.where(allow_a, 0.0, _NEG)], axis=1).astype(np.float32)
    mask0 = np.concatenate([np.where(allow_b, 0.0, _NEG),
                            np.full((128, 128), _NEG)], axis=1).astype(np.float32)
    return maskr, mask0


def _make_in_maps(states, Wq, Wk, Wv):
    states = np.ascontiguousarray(np.asarray(states, dtype=np.float32))
    B, S = states.shape[0], states.shape[1]
    x = states.reshape(B, S, _D)
    L = S // 2
    NX = L + _HALO
    wqt = np.ascontiguousarray(np.asarray(Wq, dtype=np.float32).T)
    wkt = np.ascontiguousarray(np.asarray(Wk, dtype=np.float32).T)
    wvt = np.ascontiguousarray(np.asarray(Wv, dtype=np.float32).T)
    maskr, mask0 = _masks()
    in_maps = []
    for core in range(2 * B):
        b, h = core // 2, core % 2
        xp = np.zeros((NX, _D), dtype=np.float32)
        lo = h * L - _HALO
        if lo < 0:
            xp[_HALO:] = x[b, 0:L]
        else:
            xp[:] = x[b, lo:lo + NX]
        in_maps.append({
            "xT": np.ascontiguousarray(xp.T),
            "wqt": wqt, "wkt": wkt, "wvt": wvt,
            "mask0": mask0 if h == 0 else maskr,
            "maskr": maskr,
        })
    return in_maps


def _assemble(results, B, S):
    L = S // 2
    out = np.empty((B, S, _D), dtype=np.float32)
    for core, res in enumerate(results):
        b, h = core // 2, core % 2
        out[b, h * L:(h + 1) * L] = res["out"]
    out[:, 0, :] = 0.0
    return out.reshape(B, S, _D // 2, 2)


def run(states, Wq, Wk, Wv, trace=False):
    """Run on 8 NeuronCores; returns (output, BassKernelResults)."""
    from concourse.bass_utils import run_bass_kernel_spmd

    states = np.asarray(states)
    B, S = states.shape[0], states.shape[1]
    assert B == 4 and S % 128 == 0
    nc = _get_nc(S // 2)
    in_maps = _make_in_maps(states, Wq, Wk, Wv)
    try:
        res = run_bass_kernel_spmd(nc, in_maps, core_ids=list(range(8)), trace=trace)
    except ModuleNotFoundError:
        # axon NTFF hook unavailable in this container — run untraced
        res = run_bass_kernel_spmd(nc, in_maps, core_ids=list(range(8)), trace=False)
    return _assemble(res.results, B, S), res


def kernel(states, Wq, Wk, Wv):
    out, _ = run(states, Wq, Wk, Wv, trace=False)
    return out


# revision 8
# speedup vs baseline: 585.5092x; 3.4229x over previous
"""Trainium2 Bass kernel for nn_EpisodicMemoryEfficient.

Computation (see reference): per batch, x = states reshaped [S, 512];
q/k/v = x @ {Wq,Wk,Wv}.T; chunked sliding-window attention with chunk size
C=64 where chunk i attends to chunks {i-1, i} with strict causal masking
(chunk 0 has no previous chunk), softmax over the 128-key context, out = p@v.

Sharding: pure data parallel over (batch, sequence-half) -> 8 cores.
Each core receives a pre-transposed xT [512, 4224] covering its 4096-row
shard plus a 128-row halo in front (zeros for the first half, the real
previous rows for the second half).  Weights are passed transposed and
replicated; masks are tiny per-core constants.

Device kernel layout choices:
  - matmul operands use float32r (fp32 bits, single-pass PE streaming at
    1 row/cycle for moving free dim >=256 vs 4 cycles/row for fp32;
    measured ~1.4e-4 absmax rel err per K=512 matmul). PSUM accumulation
    stays fp32.
  - qT/kT are produced directly in [d, seq] orientation (weights stationary,
    xT moving, N=512 matmuls); v in natural [seq, d] (xT stationary).
  - attention uses the transposed-scores formulation sT[j, qi] so that the
    softmax denominator comes from a matmul-with-ones column sum and the
    PV matmul needs no transpose at all (lhsT = exp(sT), rhs = v natural).
  - interior q-tiles are processed in PAIRS (qi free dim = 256) so the
    score matmuls hit the fast float32r rate; the first/last q-tiles use
    the single-tile path (N=128).
  - masking is additive (-1e30) on the [j, qi] tile before exp; the scale
    1/sqrt(512) is folded into the ACT exp instruction.
  - PSUM is evacuated with an ACT copy scaled by the per-partition 1/rowsum.
The only fully-masked row (global position 0) yields 0/0 -> NaN and is
overwritten with zeros on the host, matching the reference.
"""

import numpy as np

_D = 512
_C = 64
_HALO = 128  # two chunks of halo keeps everything 128-aligned
_NEG = -1.0e30
_SCALE = 1.0 / float(np.sqrt(_D))

_BUILD_CACHE = {}


def _build(L, repeat=1, mmdt="f32r", pair=True):
    """Build the per-core Bass kernel for a shard of L query rows.

    repeat>1 wraps the whole body in a For_i loop — used only for
    wall-clock timing (amortizes the ~100ms axon dispatch overhead).
    """
    from contextlib import ExitStack

    import concourse.mybir as mybir
    import concourse.tile as tile
    from concourse import bacc

    assert L % 128 == 0 and L >= 256
    NX = L + _HALO          # x rows held on-core (halo + shard)
    NQ = L // 128           # q-tiles
    dt = mybir.dt.float32

    nc = bacc.Bacc("TRN2", target_bir_lowering=False, debug=False, num_devices=8)
    # dtype used for matmul operands: float32r streams 1 row/cycle (vs 4 for
    # float32) when the moving free dim is >=256, at ~1.4e-4 absmax rel err
    dtm = mybir.dt.float32r if mmdt == "f32r" else mybir.dt.float32

    xT_d = nc.dram_tensor("xT", [_D, NX], dtm, kind="ExternalInput")
    wqt_d = nc.dram_tensor("wqt", [_D, _D], dtm, kind="ExternalInput")
    wkt_d = nc.dram_tensor("wkt", [_D, _D], dtm, kind="ExternalInput")
    wvt_d = nc.dram_tensor("wvt", [_D, _D], dtm, kind="ExternalInput")
    mask0_d = nc.dram_tensor("mask0", [128, 256], dt, kind="ExternalInput")
    maskr_d = nc.dram_tensor("maskr", [128, 256], dt, kind="ExternalInput")
    maskp1_d = nc.dram_tensor("maskp1", [128, 512], dt, kind="ExternalInput")
    maskp2_d = nc.dram_tensor("maskp2", [128, 256], dt, kind="ExternalInput")
    out_d = nc.dram_tensor("out", [L, _D], dt, kind="ExternalOutput")

    # projection blocks over the x column range: 512 wide, 128-wide leftover
    blocks = []
    m0 = 0
    while m0 < NX:
        mb = min(512, NX - m0)
        blocks.append((m0, mb))
        m0 += mb

    with ExitStack() as ctx:
        tc = ctx.enter_context(tile.TileContext(nc))
        const = ctx.enter_context(tc.tile_pool(name="const", bufs=1))
        xpool = ctx.enter_context(tc.tile_pool(name="xpool", bufs=2))
        qpool = ctx.enter_context(tc.tile_pool(name="qpool", bufs=3))
        kpool = ctx.enter_context(tc.tile_pool(name="kpool", bufs=3))
        vpool = ctx.enter_context(tc.tile_pool(name="vpool", bufs=3))
        spool = ctx.enter_context(tc.tile_pool(name="spool", bufs=3))
        opool = ctx.enter_context(tc.tile_pool(name="opool", bufs=3))
        ps_proj = ctx.enter_context(tc.tile_pool(name="ps_proj", bufs=2, space="PSUM"))
        ps_att = ctx.enter_context(tc.tile_pool(name="ps_att", bufs=2, space="PSUM"))
        ps_out = ctx.enter_context(tc.tile_pool(name="ps_out", bufs=2, space="PSUM"))
        ps_sum = ctx.enter_context(tc.tile_pool(name="ps_sum", bufs=1, space="PSUM"))

        # ---- constants ----
        wq_sb, wk_sb, wv_sb = [], [], []
        for c in range(4):
            for lst, dram, nm in ((wq_sb, wqt_d, "wq"), (wk_sb, wkt_d, "wk"),
                                  (wv_sb, wvt_d, "wv")):
                t_ = const.tile([128, _D], dtm, tag=f"{nm}{c}", name=f"{nm}{c}")
                nc.sync.dma_start(out=t_, in_=dram[128 * c:128 * (c + 1), :])
                lst.append(t_)
        m0_sb = const.tile([128, 256], dt, tag="m0", name="m0_sb")
        nc.sync.dma_start(out=m0_sb, in_=mask0_d[:, :])
        mr_sb = const.tile([128, 256], dt, tag="mr", name="mr_sb")
        nc.sync.dma_start(out=mr_sb, in_=maskr_d[:, :])
        mp1_sb = const.tile([128, 512], dt, tag="mp1", name="mp1_sb")
        nc.sync.dma_start(out=mp1_sb, in_=maskp1_d[:, :])
        mp2_sb = const.tile([128, 256], dt, tag="mp2", name="mp2_sb")
        nc.sync.dma_start(out=mp2_sb, in_=maskp2_d[:, :])
        # fp32 on purpose: memset can't write float32r, and the N=1 column-sum
        # matmul is illegal at float32r anyway — it runs as a plain fp32
        # matmul on the (bit-identical) exp tile instead
        ones_sb = const.tile([128, 1], dt, tag="ones", name="ones_sb")
        nc.vector.memset(ones_sb, 1.0)

        qt_tiles = {}   # (block, c) -> [128, mb] tile, cols = x-tile cols
        kt_tiles = {}
        v_tiles = {}    # x-tile index -> [128, 512] natural tile

        def kt_slice(ti, c, w=128):
            return kt_tiles[(ti // 4, c)][:, (ti % 4) * 128:(ti % 4) * 128 + w]

        def qt_slice(ti, c, w=128):
            return qt_tiles[(ti // 4, c)][:, (ti % 4) * 128:(ti % 4) * 128 + w]

        def project_block(bi):
            m0, mb = blocks[bi]
            xt = []
            for c in range(4):
                t_ = xpool.tile([128, mb], dtm, tag=f"xt{c}", name=f"xt{c}_b{bi}")
                nc.sync.dma_start(out=t_, in_=xT_d[128 * c:128 * (c + 1), m0:m0 + mb])
                xt.append(t_)
            for o in range(4):
                psq = ps_proj.tile([128, mb], dt, tag="ps_proj", name=f"psq{o}_b{bi}")
                for c in range(4):
                    nc.tensor.matmul(psq, wq_sb[c][:, 128 * o:128 * (o + 1)], xt[c],
                                     start=(c == 0), stop=(c == 3))
                qt = qpool.tile([128, mb], dtm, tag=f"qt{o}", name=f"qt{o}_b{bi}")
                nc.vector.tensor_copy(qt, psq)
                qt_tiles[(bi, o)] = qt

                psk = ps_proj.tile([128, mb], dt, tag="ps_proj", name=f"psk{o}_b{bi}")
                for c in range(4):
                    nc.tensor.matmul(psk, wk_sb[c][:, 128 * o:128 * (o + 1)], xt[c],
                                     start=(c == 0), stop=(c == 3))
                kt = kpool.tile([128, mb], dtm, tag=f"kt{o}", name=f"kt{o}_b{bi}")
                nc.vector.tensor_copy(kt, psk)
                kt_tiles[(bi, o)] = kt
            for s in range(mb // 128):
                ti = m0 // 128 + s
                psv = ps_proj.tile([128, _D], dt, tag="ps_proj", name=f"psv{s}_b{bi}")
                for c in range(4):
                    nc.tensor.matmul(psv, xt[c][:, 128 * s:128 * (s + 1)], wv_sb[c],
                                     start=(c == 0), stop=(c == 3))
                vt = vpool.tile([128, _D], dtm, tag=f"v{ti % 4}", name=f"v_t{ti}")
                nc.scalar.copy(vt, psv)
                v_tiles[ti] = vt

        def finish(t, e_lo, v_lo, e_hi, v_hi):
            """softmax denominator + PV + normalize + store for q-tile t.

            e_lo/e_hi: [j 128, qi 128] exp slices for keys in x-tiles t, t+1.
            """
            psum_t = ps_sum.tile([128, 1], dt, tag="ps_sum", name=f"pssum_t{t}")
            po = ps_out.tile([128, _D], dt, tag="ps_o", name=f"ps_o_t{t}")
            nc.tensor.matmul(psum_t, e_lo.bitcast(dt), ones_sb,
                             start=True, stop=False)
            nc.tensor.matmul(po, e_lo, v_lo, start=True, stop=False)
            nc.tensor.matmul(psum_t, e_hi.bitcast(dt), ones_sb,
                             start=False, stop=True)
            nc.tensor.matmul(po, e_hi, v_hi, start=False, stop=True)
            recip = opool.tile([128, 1], dt, tag="recip", name=f"recip_t{t}")
            nc.vector.reciprocal(recip, psum_t)
            o_sb = opool.tile([128, _D], dt, tag="o_sb", name=f"o_t{t}")
            nc.scalar.mul(o_sb, po, recip)
            nc.sync.dma_start(out=out_d[128 * t:128 * (t + 1), :], in_=o_sb)

        def attend_single(t):
            # q = x-tile t+1; keys region B = x-tile t+1, region A = x-tile t
            ps_s = ps_att.tile([128, 256], dt, tag="ps_p1", name=f"ps_s_t{t}")
            for c in range(4):  # region B -> cols 0:128 of ps_s
                nc.tensor.matmul(ps_s[:, 0:128], kt_slice(t + 1, c),
                                 qt_slice(t + 1, c),
                                 start=(c == 0), stop=(c == 3))
            for c in range(4):  # region A -> cols 128:256
                nc.tensor.matmul(ps_s[:, 128:256], kt_slice(t, c),
                                 qt_slice(t + 1, c),
                                 start=(c == 0), stop=(c == 3))
            s_sb = spool.tile([128, 256], dt, tag="s_sb", name=f"s_t{t}")
            nc.vector.tensor_add(s_sb, ps_s, m0_sb if t == 0 else mr_sb)
            e_sb = spool.tile([128, 256], dtm, tag="e_sb", name=f"e_t{t}")
            nc.scalar.activation(e_sb, s_sb, mybir.ActivationFunctionType.Exp,
                                 scale=_SCALE)
            finish(t, e_sb[:, 128:256], v_tiles[t], e_sb[:, 0:128], v_tiles[t + 1])

        def attend_pair(u):
            """q-tiles t1 = 2u-1 and t2 = 2u; q x-tiles 2u, 2u+1 (contiguous,
            always within one block). Key x-tiles: 2u-1 (R0), 2u (R1), 2u+1
            (R2). All score matmuls run at the fast N=256 rate; out-of-window
            (qi, j) combinations are masked additively."""
            t1, t2 = 2 * u - 1, 2 * u
            qx = 2 * u  # first q x-tile
            p1 = ps_att.tile([128, 512], dt, tag="ps_p1", name=f"ps_p1_u{u}")
            p2 = ps_att.tile([128, 256], dt, tag="ps_p2", name=f"ps_p2_u{u}",
                             bufs=1)
            for c in range(4):  # R0: keys x-tile 2u-1
                nc.tensor.matmul(p1[:, 0:256], kt_slice(2 * u - 1, c),
                                 qt_slice(qx, c, 256),
                                 start=(c == 0), stop=(c == 3))
            for c in range(4):  # R1: keys x-tile 2u
                nc.tensor.matmul(p1[:, 256:512], kt_slice(2 * u, c),
                                 qt_slice(qx, c, 256),
                                 start=(c == 0), stop=(c == 3))
            for c in range(4):  # R2: keys x-tile 2u+1
                nc.tensor.matmul(p2, kt_slice(2 * u + 1, c),
                                 qt_slice(qx, c, 256),
                                 start=(c == 0), stop=(c == 3))
            s1 = spool.tile([128, 512], dt, tag="s1", name=f"s1_u{u}")
            nc.vector.tensor_add(s1, p1, mp1_sb)
            s2 = spool.tile([128, 256], dt, tag="s2", name=f"s2_u{u}")
            nc.vector.tensor_add(s2, p2, mp2_sb)
            e1 = spool.tile([128, 512], dtm, tag="e1", name=f"e1_u{u}")
            nc.scalar.activation(e1, s1, mybir.ActivationFunctionType.Exp,
                                 scale=_SCALE)
            e2 = spool.tile([128, 256], dtm, tag="e2", name=f"e2_u{u}")
            nc.scalar.activation(e2, s2, mybir.ActivationFunctionType.Exp,
                                 scale=_SCALE)
            finish(t1, e1[:, 0:128], v_tiles[t1], e1[:, 256:384], v_tiles[t1 + 1])
            finish(t2, e1[:, 384:512], v_tiles[t2], e2[:, 128:256], v_tiles[t2 + 1])

        # attention units in dependency order: unit ready after its newest
        # x-tile's block is projected
        units = []  # (required_block, emit_fn)
        if pair:
            units.append(((0 + 1) // 4, lambda: attend_single(0)))
            for u in range(1, NQ // 2):
                units.append(((2 * u + 1) // 4, lambda u=u: attend_pair(u)))
            units.append((NQ // 4, lambda: attend_single(NQ - 1)))
        else:
            for t in range(NQ):
                units.append(((t + 1) // 4, lambda t=t: attend_single(t)))

        def body():
            ui = 0
            for bi in range(len(blocks)):
                project_block(bi)
                while ui < len(units) and units[ui][0] == bi:
                    units[ui][1]()
                    ui += 1
            assert ui == len(units), (ui, len(units))

        if repeat == 1:
            body()
        else:
            with tc.For_i(0, repeat, 1,
                          hint_engines=(mybir.EngineType.PE, mybir.EngineType.DVE,
                                        mybir.EngineType.Activation,
                                        mybir.EngineType.SP)):
                body()

    nc.compile()
    return nc


def _get_nc(L):
    if L not in _BUILD_CACHE:
        _BUILD_CACHE[L] = _build(L)
    return _BUILD_CACHE[L]


def _masks():
    """Single-tile masks [j 128, qi 128 | qi 128] in the transposed-scores
    orientation. Region B (cols 0:128): keys in the query's own x-tile —
    strict causal p < qi. Region A (cols 128:256): the preceding x-tile —
    only its second chunk (p >= 64) and only for the first-chunk queries
    (qi < 64). mask0 is the no-previous-chunk variant for global chunk 0."""
    p = np.arange(128)[:, None]
    q = np.arange(128)[None, :]
    allow_b = p < q
    allow_a = (p >= 64) & (q < 64)
    mb_ = np.where(allow_b, 0.0, _NEG)
    ma_ = np.where(allow_a, 0.0, _NEG)
    neg = np.full((128, 128), _NEG)
    maskr = np.concatenate([mb_, ma_], axis=1).astype(np.float32)
    mask0 = np.concatenate([mb_, neg], axis=1).astype(np.float32)
    # pair masks: qi = [q-tile t1 | q-tile t2]; R0 keys = x-tile t1,
    # R1 keys = x-tile t1+1 (= t2), R2 keys = x-tile t2+1
    maskp1 = np.concatenate([np.concatenate([ma_, neg], axis=1),     # R0
                             np.concatenate([mb_, ma_], axis=1)],    # R1
                            axis=1).astype(np.float32)               # [128,512]
    maskp2 = np.concatenate([neg, mb_], axis=1).astype(np.float32)   # R2
    return maskr, mask0, maskp1, maskp2


def _make_in_maps(states, Wq, Wk, Wv):
    states = np.ascontiguousarray(np.asarray(states, dtype=np.float32))
    B, S = states.shape[0], states.shape[1]
    x = states.reshape(B, S, _D)
    L = S // 2
    NX = L + _HALO
    wqt = np.ascontiguousarray(np.asarray(Wq, dtype=np.float32).T)
    wkt = np.ascontiguousarray(np.asarray(Wk, dtype=np.float32).T)
    wvt = np.ascontiguousarray(np.asarray(Wv, dtype=np.float32).T)
    maskr, mask0, maskp1, maskp2 = _masks()
    in_maps = []
    for core in range(2 * B):
        b, h = core // 2, core % 2
        xp = np.zeros((NX, _D), dtype=np.float32)
        lo = h * L - _HALO
        if lo < 0:
            xp[_HALO:] = x[b, 0:L]
        else:
            xp[:] = x[b, lo:lo + NX]
        in_maps.append({
            "xT": np.ascontiguousarray(xp.T),
            "wqt": wqt, "wkt": wkt, "wvt": wvt,
            "mask0": mask0 if h == 0 else maskr,
            "maskr": maskr, "maskp1": maskp1, "maskp2": maskp2,
        })
    return in_maps


def _assemble(results, B, S):
    L = S // 2
    out = np.empty((B, S, _D), dtype=np.float32)
    for core, res in enumerate(results):
        b, h = core // 2, core % 2
        out[b, h * L:(h + 1) * L] = res["out"]
    out[:, 0, :] = 0.0
    return out.reshape(B, S, _D // 2, 2)


def run(states, Wq, Wk, Wv, trace=False):
    """Run on 8 NeuronCores; returns (output, BassKernelResults)."""
    from concourse.bass_utils import run_bass_kernel_spmd

    states = np.asarray(states)
    B, S = states.shape[0], states.shape[1]
    assert B == 4 and S % 128 == 0
    nc = _get_nc(S // 2)
    in_maps = _make_in_maps(states, Wq, Wk, Wv)
    try:
        res = run_bass_kernel_spmd(nc, in_maps, core_ids=list(range(8)), trace=trace)
    except ModuleNotFoundError:
        # axon NTFF hook unavailable in this container — run untraced
        res = run_bass_kernel_spmd(nc, in_maps, core_ids=list(range(8)), trace=False)
    return _assemble(res.results, B, S), res


def kernel(states, Wq, Wk, Wv):
    out, _ = run(states, Wq, Wk, Wv, trace=False)
    return out
